# revision 9
# baseline (speedup 1.0000x reference)
"""Multi-head attention (B=2, T=2048, DIM=2048, NH=16, HD=128, partial RoPE)
on 8 Trainium2 NeuronCores.

Sharding (Megatron-style): core c handles batch b = c // 4 and head group
g = c % 4 (heads 4g..4g+3, i.e. 512 of the 2048 q/k/v channels). Each core
computes its heads' attention output and the partial output projection
y_heads @ wo[:, cols].T -> [T, DIM]; the host sums the 4 partials per batch.

Single fused pipeline per core (no DRAM scratch, bf16 datapath, fp32 accum):

  Phase A (x-stream + projections): xT streams in once as bf16 d-chunks; four
  waves each accumulate 4 v t-tiles (PSUM, 1 bank each) + one full q/k head
  row (two [128,1024] halves, 2 banks each) with the d-loop outermost, so the
  PE consumes each arriving x chunk across 8 open PSUM banks.  v tiles drain
  PSUM->SBUF bf16 on Pool; q/k halves drain on ACT (identity copy) and get
  RoPE applied in-place on DVE.  After wave A4: v complete, heads 0-1 ready.

  Phase B: remaining q/k rows (heads 2-3) interleave with attention units for
  heads 0-1.  An attention unit is one (head, 512-query block): per key tile,
  an S^T matmul -> exp on ACT (scale=1/sqrt(hd), no max subtraction; logits
  O(5)) -> bf16 P^T -> AV accumulation into PSUM, with one projection d-step
  between attention steps so the PE never waits on ACT.  Softmax sums never
  touch the PE: an eager pair/quad/eighth add tree alternates DVE (bf16) and
  Pool (fp32), finishing with partition_all_reduce on Pool; reciprocal +
  normalize-mul on DVE write yT bf16.

  Phase C: attention for heads 2-3, with the output projection of query
  block b-1 (yT^T @ woT, 4-head PSUM accumulation) interleaved one matmul
  per attention step; results drain on Pool and DMA out on SP.

  DMA queues: SP carries the xT stream and output stores; Pool carries
  weights/tables (wv chunks interleaved between the early wq/wk stationary
  loads; heads 2-3 stationaries and woT loaded just-in-time).  All matmuls
  run bf16 (full PE rate, fp32 PSUM accumulation).
"""

import math

import numpy as np

B, T, DIM, NH = 2, 2048, 2048, 16
HD = DIM // NH          # 128
P = 128
NHC = 4                 # heads per core
CC = NHC * HD           # 512 channels per core
ND = DIM // P           # 16 d-chunks
NT = T // P             # 16 token tiles
HBLK = 1024             # q/k projection half width
ABLK = 512              # attention query block
NAB = T // ABLK         # 4
NTB = ABLK // P         # t-tiles per attention block (4)
SOFTMAX_SCALE = 1.0 / math.sqrt(HD)

_CACHED = {}


def _rope_tables():
    """tblc [64, T] = cos(invfreq_j * t); tbls [64, T]: rows 0:32 = -sin_j,
    rows 32:64 = +sin_j (swap-term). Used as
      tmp[0:32]  = copy(q[32:64]);  tmp[32:64] = copy(q[0:32])
      tmp[0:64] *= tbls;  q[0:64] *= tblc;  q[0:64] += tmp
    """
    import ml_dtypes

    inv_freq = 1.0 / (10000.0 ** (np.arange(0, HD, 2, dtype=np.float64) / HD))
    t = np.arange(T, dtype=np.float64)
    ang = np.outer(inv_freq, t)              # [64, T]
    tblc = np.cos(ang).astype(ml_dtypes.bfloat16)
    sin = np.sin(ang)
    tbls = np.empty((64, T), dtype=np.float64)
    tbls[0:32] = -sin[0:32]
    tbls[32:64] = sin[32:64]
    return tblc, tbls.astype(ml_dtypes.bfloat16)


def _build():
    import concourse.bacc as bacc
    import concourse.bass_isa as bass_isa
    import concourse.mybir as mybir
    import concourse.tile as tile

    FP32 = mybir.dt.float32
    BF16 = mybir.dt.bfloat16
    EXP = mybir.ActivationFunctionType.Exp
    COPY = mybir.ActivationFunctionType.Copy
    ADD = bass_isa.ReduceOp.add

    nc = bacc.Bacc("TRN2", target_bir_lowering=False, debug=False, num_devices=8)
    xT = nc.declare_dram_parameter("xT", [DIM, T], BF16, isOutput=False)
    wqT = nc.declare_dram_parameter("wqT", [NHC, DIM, HD], BF16, isOutput=False)
    wkT = nc.declare_dram_parameter("wkT", [NHC, DIM, HD], BF16, isOutput=False)
    wvT = nc.declare_dram_parameter("wvT", [DIM, CC], BF16, isOutput=False)
    woT = nc.declare_dram_parameter("woT", [CC, DIM], BF16, isOutput=False)
    tblcp = nc.declare_dram_parameter("tblc", [64, T], BF16, isOutput=False)
    tblsp = nc.declare_dram_parameter("tbls", [64, T], BF16, isOutput=False)
    out = nc.declare_dram_parameter("out", [T, DIM], FP32, isOutput=True)

    with tile.TileContext(nc) as tc:
        # ------------- SBUF pools (alloc order = reverse release order) ----
        const_pool = tc.alloc_tile_pool(name="const", bufs=1)
        qk_pool = tc.alloc_tile_pool(name="qk", bufs=1)
        vh_pool = tc.alloc_tile_pool(name="vh", bufs=1)
        yT_pool = tc.alloc_tile_pool(name="yT", bufs=1)
        rope_pool = tc.alloc_tile_pool(name="rope", bufs=2)
        pt_pool = tc.alloc_tile_pool(name="pt", bufs=4)
        s1_pool = tc.alloc_tile_pool(name="s1", bufs=4)
        s2_pool = tc.alloc_tile_pool(name="s2", bufs=4)
        s3_pool = tc.alloc_tile_pool(name="s3", bufs=3)
        rs_pool = tc.alloc_tile_pool(name="rs", bufs=2)
        # phase-scoped pools (released end of phase B, LIFO)
        wt_pool = tc.alloc_tile_pool(name="wt", bufs=4)
        wv_pool = tc.alloc_tile_pool(name="wv", bufs=1)
        xT_pool = tc.alloc_tile_pool(name="xT", bufs=1)

        tblc = const_pool.tile([64, T], BF16)
        nc.gpsimd.dma_start(tblc[:], tblcp[:, :])
        tbls = const_pool.tile([64, T], BF16)
        nc.gpsimd.dma_start(tbls[:], tblsp[:, :])

        def load_wt(kind, h):
            src = wqT if kind == "q" else wkT
            wt = wt_pool.tile([P, ND, HD], BF16, name=f"wt{kind}{h}", tag="wt")
            nc.gpsimd.dma_start(wt[:], src[h].rearrange("(n p) c -> p n c", p=P))
            return wt

        # wv moving tiles (one per d-chunk), interleaved with the early wt
        # loads on the Pool queue so wv0 lands ~2us in
        wvs = [None] * ND

        def load_wv(lo, hi):
            for d in range(lo, hi):
                w_ = wv_pool.tile([P, CC], BF16, name=f"wv{d}", tag=f"wv{d}")
                nc.gpsimd.dma_start(w_[:], wvT[d * P:(d + 1) * P, :])
                wvs[d] = w_

        wts = {}
        wts[("k", 0)] = load_wt("k", 0)
        load_wv(0, 4)
        wts[("q", 0)] = load_wt("q", 0)
        load_wv(4, 8)
        wts[("k", 1)] = load_wt("k", 1)
        load_wv(8, 12)
        wts[("q", 1)] = load_wt("q", 1)
        load_wv(12, 16)

        # xT resident d-chunk tiles, streamed on SP
        xTs = []
        for d in range(ND):
            t_ = xT_pool.tile([P, T], BF16, name=f"xt{d}", tag=f"xt{d}")
            nc.sync.dma_start(t_[:], xT[d * P:(d + 1) * P, :])
            xTs.append(t_)

        # q/k head rows [hd, T] bf16 (RoPE applied in place)
        qh = [qk_pool.tile([P, T], BF16, name=f"qh{h}", tag=f"qh{h}")
              for h in range(NHC)]
        kh = [qk_pool.tile([P, T], BF16, name=f"kh{h}", tag=f"kh{h}")
              for h in range(NHC)]

        # v t-tiles [128 keys, 512 cc] bf16
        vhs = [vh_pool.tile([P, CC], BF16, name=f"vh{ti}", tag=f"vh{ti}")
               for ti in range(NT)]

        # attention output yT [hd, h, T] bf16
        yT4 = yT_pool.tile([P, NHC, T], BF16)

        # ------------- helpers --------------------------------------------
        def rope_inplace(dst, half):
            """Apply partial RoPE to dst[0:64, half*HBLK:(half+1)*HBLK]."""
            c0 = half * HBLK
            sl = slice(c0, c0 + HBLK)
            tmp = rope_pool.tile([64, HBLK], BF16, name=f"rp{half}", tag="rp")
            nc.vector.tensor_copy(out=tmp[0:32], in_=dst[32:64, sl])
            nc.vector.tensor_copy(out=tmp[32:64], in_=dst[0:32, sl])
            nc.vector.tensor_mul(out=tmp[:], in0=tmp[:], in1=tbls[:, sl])
            nc.vector.tensor_mul(out=dst[0:64, sl], in0=dst[0:64, sl],
                                 in1=tblc[:, sl])
            nc.vector.tensor_add(out=dst[0:64, sl], in0=dst[0:64, sl],
                                 in1=tmp[:])

        def qk_row_drain(qp, kind, h, half):
            """PSUM [128, HBLK] -> bf16 row half + RoPE."""
            dst = qh[h] if kind == "q" else kh[h]
            c0 = half * HBLK
            nc.scalar.activation(out=dst[:, c0:c0 + HBLK], in_=qp[:],
                                 func=COPY)
            rope_inplace(dst, half)

        # attention unit: one (head, query-block) of ABLK queries
        class AttnUnit:
            def __init__(self, h, b, st_pool, yacc_pool):
                self.h, self.b = h, b
                self.q0 = b * ABLK
                self.st_pool = st_pool
                self.yacc = yacc_pool.tile([P, ABLK], FP32,
                                           name=f"ya{h}_{b}", tag="yacc")
                self.pts = []
                self.s1 = []
                self.s2 = []
                self.s3 = []

            def step(self, tk):
                h, b, q0 = self.h, self.b, self.q0
                st = self.st_pool.tile([P, ABLK], FP32,
                                       name=f"st{h}_{b}_{tk}", tag="st")
                nc.tensor.matmul(st[:], kh[h][:, tk * P:(tk + 1) * P],
                                 qh[h][:, q0:q0 + ABLK], start=True, stop=True)
                pt = pt_pool.tile([P, ABLK], BF16,
                                  name=f"pt{h}_{b}_{tk}", tag="pt")
                nc.scalar.activation(out=pt[:], in_=st[:], func=EXP,
                                     scale=SOFTMAX_SCALE)
                self.pts.append(pt)
                # eager softmax-sum tree (none of it on the PE):
                # pairs on DVE (bf16), quads on Pool (fp32), eighths on DVE
                if tk % 2 == 1:
                    s1 = s1_pool.tile([P, ABLK], BF16,
                                      name=f"s1_{h}_{b}_{tk}", tag="s1")
                    nc.vector.tensor_add(out=s1[:], in0=self.pts[tk - 1][:],
                                         in1=pt[:])
                    self.s1.append(s1)
                if tk % 4 == 3:
                    s2 = s2_pool.tile([P, ABLK], FP32,
                                      name=f"s2_{h}_{b}_{tk}", tag="s2")
                    nc.gpsimd.tensor_add(out=s2[:], in0=self.s1[-2][:],
                                         in1=self.s1[-1][:])
                    self.s2.append(s2)
                if tk % 8 == 7:
                    s3 = s3_pool.tile([P, ABLK], FP32,
                                      name=f"s3_{h}_{b}_{tk}", tag="s3")
                    nc.vector.tensor_add(out=s3[:], in0=self.s2[-2][:],
                                         in1=self.s2[-1][:])
                    self.s3.append(s3)

            def av(self, tk):
                h = self.h
                nc.tensor.matmul(self.yacc[:],
                                 vhs[tk][:, h * HD:(h + 1) * HD],
                                 self.pts[tk][:],
                                 start=(tk == 0), stop=(tk == NT - 1))

            def finish(self):
                h, b = self.h, self.b
                s4 = rs_pool.tile([P, ABLK], FP32, name=f"s4_{h}_{b}", tag="s4")
                nc.gpsimd.tensor_add(out=s4[:], in0=self.s3[0][:],
                                     in1=self.s3[1][:])
                rsum = rs_pool.tile([P, ABLK], FP32, name=f"sr_{h}_{b}",
                                    tag="sr")
                nc.gpsimd.partition_all_reduce(rsum[:], s4[:], channels=P,
                                               reduce_op=ADD)
                rs = rs_pool.tile([P, ABLK], FP32, name=f"rs_{h}_{b}", tag="rs")
                nc.vector.reciprocal(out=rs[:], in_=rsum[:])
                dst = yT4[:, h, self.q0:self.q0 + ABLK]
                nc.vector.tensor_mul(out=dst, in0=self.yacc[:], in1=rs[:])

        # ------------- phase A: x-stream + v + heads 0-1 ------------------
        qkp_pool = tc.alloc_tile_pool(name="qkp", bufs=2, space="PSUM",
                                      side="right")
        with tc.tile_pool(name="vps", bufs=4, space="PSUM",
                          side="right") as vps_pool:
            a_rows = [("k", 0), ("q", 0), ("k", 1), ("q", 1)]
            for w, (kind, h) in enumerate(a_rows):
                qp = [qkp_pool.tile([P, HBLK], FP32, name=f"qp{kind}{h}{hf}",
                                    tag="qkp") for hf in range(2)]
                vt = [vps_pool.tile([P, CC], FP32, name=f"vp{w}_{j}",
                                    tag="vps") for j in range(4)]
                wt = wts[(kind, h)]
                for d in range(ND):
                    for hf in range(2):
                        for c in range(2):
                            c0 = hf * HBLK + c * ABLK
                            nc.tensor.matmul(
                                qp[hf][:, c * ABLK:(c + 1) * ABLK],
                                wt[:, d, :], xTs[d][:, c0:c0 + ABLK],
                                start=(d == 0), stop=(d == ND - 1))
                    for j in range(4):
                        ti = 4 * w + j
                        nc.tensor.matmul(
                            vt[j][:], xTs[d][:, ti * P:(ti + 1) * P],
                            wvs[d][:], start=(d == 0), stop=(d == ND - 1))
                for hf in range(2):
                    qk_row_drain(qp[hf], kind, h, hf)
                for j in range(4):
                    nc.scalar.activation(out=vhs[4 * w + j][:], in_=vt[j][:],
                                         func=COPY)

        # ------------- phases B & C ---------------------------------------
        with (
            tc.tile_pool(name="st", bufs=2, space="PSUM") as st_pool,
            tc.tile_pool(name="yacc", bufs=2, space="PSUM") as yacc_pool,
        ):  # left-side PSUM, coexists with right-side qkp then op
            # phase B: heads 2-3 projections interleaved with attention on
            # heads 0-1; one projection d-step (one [128,1024] half matmul)
            # per attention step
            b_rows = [("k", 2), ("q", 2), ("k", 3), ("q", 3)]
            b_units = [(0, 0), (0, 1), (0, 2), (0, 3),
                       (1, 0), (1, 1), (1, 2), (1, 3)]
            for kind, h in b_rows:
                # just-in-time stationary prefetch (slot from phase A free)
                wts[(kind, h)] = load_wt(kind, h)
            for w, (kind, h) in enumerate(b_rows):
                wt = wts[(kind, h)]
                for hf in range(2):
                    u = AttnUnit(*b_units[2 * w + hf], st_pool, yacc_pool)
                    qp = qkp_pool.tile([P, HBLK], FP32,
                                       name=f"qp{kind}{h}{hf}", tag="qkp")
                    for i in range(ND):
                        for c in range(2):
                            c0 = hf * HBLK + c * ABLK
                            nc.tensor.matmul(
                                qp[:, c * ABLK:(c + 1) * ABLK],
                                wt[:, i, :], xTs[i][:, c0:c0 + ABLK],
                                start=(i == 0), stop=(i == ND - 1))
                        u.step(i)
                        if i > 0:
                            u.av(i - 1)
                    qk_row_drain(qp, kind, h, hf)
                    u.av(NT - 1)
                    u.finish()

            qkp_pool.release()
            xT_pool.release()
            wv_pool.release()
            wt_pool.release()
            # (xT/wv/wt are the top of the left SBUF stack, popped LIFO)

            # phase C: attention heads 2-3; oproj of block b-1 interleaved
            # one matmul per attention step
            wo_pool = tc.alloc_tile_pool(name="wo", bufs=1)
            wot = wo_pool.tile([P, NHC, DIM], BF16)
            nc.gpsimd.dma_start(wot[:], woT.rearrange("(h p) c -> p h c", p=P))
            ostage_pool = tc.alloc_tile_pool(name="ostage", bufs=2)

            with tc.tile_pool(name="op", bufs=2, space="PSUM",
                              side="right") as op_pool:
                op_state = {}

                def oproj_step(ti, half, hh, c):
                    """Emit matmul (hh, c) of oproj unit (ti, half)."""
                    if hh == 0 and c == 0:
                        op_state[(ti, half)] = op_pool.tile(
                            [P, HBLK], FP32, name=f"op{ti}_{half}", tag="op")
                    op = op_state[(ti, half)]
                    c0 = half * HBLK + c * ABLK
                    nc.tensor.matmul(op[:, c * ABLK:(c + 1) * ABLK],
                                     yT4[:, hh, ti * P:(ti + 1) * P],
                                     wot[:, hh, c0:c0 + ABLK],
                                     start=(hh == 0), stop=(hh == NHC - 1))
                    if hh == NHC - 1 and c == 1:
                        c0 = half * HBLK
                        os_ = ostage_pool.tile([P, HBLK], FP32,
                                               name=f"os{ti}_{half}", tag="os")
                        nc.vector.tensor_copy(out=os_[:], in_=op[:])
                        nc.sync.dma_start(
                            out[ti * P:(ti + 1) * P, c0:c0 + HBLK], os_[:])

                def oproj_block(b):
                    # 8 units x 8 matmuls for query block b, as a step list
                    steps = []
                    for j in range(NTB):
                        for hf in range(2):
                            for hh in range(NHC):
                                for c in range(2):
                                    steps.append((b * NTB + j, hf, hh, c))
                    return steps

                for b in range(NAB):
                    osteps = oproj_block(b - 1) if b > 0 else []
                    oi = 0
                    for h in (2, 3):
                        u = AttnUnit(h, b, st_pool, yacc_pool)
                        for i in range(ND):
                            u.step(i)
                            if i > 0:
                                u.av(i - 1)
                            for _ in range(2):
                                if oi < len(osteps):
                                    oproj_step(*osteps[oi])
                                    oi += 1
                        u.av(NT - 1)
                        u.finish()
                    while oi < len(osteps):
                        oproj_step(*osteps[oi])
                        oi += 1
                # tail: oproj for the last block
                for step in oproj_block(NAB - 1):
                    oproj_step(*step)

            ostage_pool.release()
            wo_pool.release()

        rs_pool.release()
        s3_pool.release()
        s2_pool.release()
        s1_pool.release()
        pt_pool.release()
        rope_pool.release()
        yT_pool.release()
        vh_pool.release()
        qk_pool.release()
        const_pool.release()

    nc.finalize()
    return nc


def _get_nc():
    if "nc" not in _CACHED:
        _CACHED["nc"] = _build()
    return _CACHED["nc"]


def _in_maps(x, wq, wk, wv, wo):
    import ml_dtypes

    BF = ml_dtypes.bfloat16
    tblc, tbls = _rope_tables()
    in_maps = []
    for core in range(8):
        b, g = divmod(core, 4)
        rows = slice(g * CC, (g + 1) * CC)
        in_maps.append({
            "xT": np.ascontiguousarray(x[b].T).astype(BF),
            "wqT": np.ascontiguousarray(
                wq[rows].reshape(NHC, HD, DIM).transpose(0, 2, 1)).astype(BF),
            "wkT": np.ascontiguousarray(
                wk[rows].reshape(NHC, HD, DIM).transpose(0, 2, 1)).astype(BF),
            "wvT": np.ascontiguousarray(wv[rows].T).astype(BF),
            "woT": np.ascontiguousarray(wo[:, rows].T).astype(BF),
            "tblc": tblc,
            "tbls": tbls,
        })
    return in_maps


def kernel(x, wq, wk, wv, wo):
    from concourse.bass_utils import run_bass_kernel_spmd

    x = np.asarray(x, dtype=np.float32)
    wq = np.asarray(wq, dtype=np.float32)
    wk = np.asarray(wk, dtype=np.float32)
    wv = np.asarray(wv, dtype=np.float32)
    wo = np.asarray(wo, dtype=np.float32)

    nc = _get_nc()
    res = run_bass_kernel_spmd(nc, _in_maps(x, wq, wk, wv, wo),
                               core_ids=list(range(8)))
    out = np.empty((B, T, DIM), dtype=np.float32)
    for b in range(B):
        out[b] = (res.results[4 * b + 0]["out"] + res.results[4 * b + 1]["out"]
                  + res.results[4 * b + 2]["out"] + res.results[4 * b + 3]["out"])
    return out


# revision 11
# speedup vs baseline: 1.0223x; 1.0223x over previous
"""Multi-head attention (B=2, T=2048, DIM=2048, NH=16, HD=128, partial RoPE)
on 8 Trainium2 NeuronCores.

Sharding (Megatron-style): core c handles batch b = c // 4 and head group
g = c % 4 (heads 4g..4g+3, i.e. 512 of the 2048 q/k/v channels). Each core
computes its heads' attention output and the partial output projection
y_heads @ wo[:, cols].T -> [T, DIM]; the host sums the 4 partials per batch.

Single fused pipeline per core (no DRAM scratch, bf16 datapath, fp32 accum):

  Phase A (x-stream + projections): xT streams in once as bf16 d-chunks; four
  waves each accumulate 4 v t-tiles (PSUM, 1 bank each) + one full q/k head
  row (two [128,1024] halves, 2 banks each) with the d-loop outermost, so the
  PE consumes each arriving x chunk across 8 open PSUM banks.  v tiles drain
  PSUM->SBUF bf16 on Pool; q/k halves drain on ACT (identity copy) and get
  RoPE applied in-place on DVE.  After wave A4: v complete, heads 0-1 ready.

  Phase B: remaining q/k rows (heads 2-3) interleave with attention units for
  heads 0-1.  An attention unit is one (head, 512-query block): per key tile,
  an S^T matmul -> exp on ACT (scale=1/sqrt(hd), no max subtraction; logits
  O(5)) -> bf16 P^T -> AV accumulation into PSUM, with one projection d-step
  between attention steps so the PE never waits on ACT.  Softmax sums never
  touch the PE: an eager pair/quad/eighth add tree alternates DVE (bf16) and
  Pool (fp32), finishing with partition_all_reduce on Pool; reciprocal +
  normalize-mul on DVE write yT bf16.

  Phase C: attention for heads 2-3, with the output projection of query
  block b-1 (yT^T @ woT, 4-head PSUM accumulation) interleaved one matmul
  per attention step; results drain on Pool and DMA out on SP.

  DMA queues: SP carries the xT stream and output stores; Pool carries
  weights/tables (wv chunks interleaved between the early wq/wk stationary
  loads; heads 2-3 stationaries and woT loaded just-in-time).  All matmuls
  run bf16 (full PE rate, fp32 PSUM accumulation).
"""

import math

import numpy as np

B, T, DIM, NH = 2, 2048, 2048, 16
HD = DIM // NH          # 128
P = 128
NHC = 4                 # heads per core
CC = NHC * HD           # 512 channels per core
ND = DIM // P           # 16 d-chunks
NT = T // P             # 16 token tiles
HBLK = 1024             # q/k projection half width
ABLK = 512              # attention query block
NAB = T // ABLK         # 4
NTB = ABLK // P         # t-tiles per attention block (4)
SOFTMAX_SCALE = 1.0 / math.sqrt(HD)

_CACHED = {}


def _rope_tables():
    """tblc [64, T] = cos(invfreq_j * t); tbls [64, T]: rows 0:32 = -sin_j,
    rows 32:64 = +sin_j (swap-term). Used as
      tmp[0:32]  = copy(q[32:64]);  tmp[32:64] = copy(q[0:32])
      tmp[0:64] *= tbls;  q[0:64] *= tblc;  q[0:64] += tmp
    """
    import ml_dtypes

    inv_freq = 1.0 / (10000.0 ** (np.arange(0, HD, 2, dtype=np.float64) / HD))
    t = np.arange(T, dtype=np.float64)
    ang = np.outer(inv_freq, t)              # [64, T]
    tblc = np.cos(ang).astype(ml_dtypes.bfloat16)
    sin = np.sin(ang)
    tbls = np.empty((64, T), dtype=np.float64)
    tbls[0:32] = -sin[0:32]
    tbls[32:64] = sin[32:64]
    return tblc, tbls.astype(ml_dtypes.bfloat16)


def _build():
    import concourse.bacc as bacc
    import concourse.bass_isa as bass_isa
    import concourse.mybir as mybir
    import concourse.tile as tile

    FP32 = mybir.dt.float32
    BF16 = mybir.dt.bfloat16
    EXP = mybir.ActivationFunctionType.Exp
    COPY = mybir.ActivationFunctionType.Copy
    ADD = bass_isa.ReduceOp.add

    nc = bacc.Bacc("TRN2", target_bir_lowering=False, debug=False, num_devices=8)
    xT = nc.declare_dram_parameter("xT", [DIM, T], BF16, isOutput=False)
    wqT = nc.declare_dram_parameter("wqT", [NHC, P, ND, HD], BF16,
                                    isOutput=False)
    wkT = nc.declare_dram_parameter("wkT", [NHC, P, ND, HD], BF16,
                                    isOutput=False)
    wvT = nc.declare_dram_parameter("wvT", [DIM, CC], BF16, isOutput=False)
    woT = nc.declare_dram_parameter("woT", [P, NHC, DIM], BF16, isOutput=False)
    tblcp = nc.declare_dram_parameter("tblc", [64, T], BF16, isOutput=False)
    tblsp = nc.declare_dram_parameter("tbls", [64, T], BF16, isOutput=False)
    out = nc.declare_dram_parameter("out", [T, DIM], FP32, isOutput=True)

    with tile.TileContext(nc) as tc:
        # ------------- SBUF pools (alloc order = reverse release order) ----
        const_pool = tc.alloc_tile_pool(name="const", bufs=1)
        qk_pool = tc.alloc_tile_pool(name="qk", bufs=1)
        vh_pool = tc.alloc_tile_pool(name="vh", bufs=1)
        yT_pool = tc.alloc_tile_pool(name="yT", bufs=1)
        rope_pool = tc.alloc_tile_pool(name="rope", bufs=2)
        pt_pool = tc.alloc_tile_pool(name="pt", bufs=4)
        s1_pool = tc.alloc_tile_pool(name="s1", bufs=4)
        s2_pool = tc.alloc_tile_pool(name="s2", bufs=4)
        s3_pool = tc.alloc_tile_pool(name="s3", bufs=3)
        rs_pool = tc.alloc_tile_pool(name="rs", bufs=2)
        # phase-scoped pools (released end of phase B, LIFO)
        wt_pool = tc.alloc_tile_pool(name="wt", bufs=4)
        wv_pool = tc.alloc_tile_pool(name="wv", bufs=1)
        xT_pool = tc.alloc_tile_pool(name="xT", bufs=1)

        def load_wt(kind, h):
            src = wqT if kind == "q" else wkT
            wt = wt_pool.tile([P, ND, HD], BF16, name=f"wt{kind}{h}", tag="wt")
            nc.gpsimd.dma_start(wt[:], src[h])
            return wt

        # wv moving tiles (one per d-chunk), interleaved with the early wt
        # loads on the Pool queue so wv0 lands ~2us in
        wvs = [None] * ND

        def load_wv(lo, hi):
            for d in range(lo, hi):
                w_ = wv_pool.tile([P, CC], BF16, name=f"wv{d}", tag=f"wv{d}")
                nc.gpsimd.dma_start(w_[:], wvT[d * P:(d + 1) * P, :])
                wvs[d] = w_

        wts = {}
        wts[("k", 0)] = load_wt("k", 0)
        load_wv(0, 4)
        wts[("q", 0)] = load_wt("q", 0)
        load_wv(4, 8)
        wts[("k", 1)] = load_wt("k", 1)
        load_wv(8, 12)
        wts[("q", 1)] = load_wt("q", 1)
        load_wv(12, 16)
        # RoPE tables load after the critical-path weights
        tblc = const_pool.tile([64, T], BF16)
        nc.gpsimd.dma_start(tblc[:], tblcp[:, :])
        tbls = const_pool.tile([64, T], BF16)
        nc.gpsimd.dma_start(tbls[:], tblsp[:, :])

        # xT resident d-chunk tiles, streamed on SP
        xTs = []
        for d in range(ND):
            t_ = xT_pool.tile([P, T], BF16, name=f"xt{d}", tag=f"xt{d}")
            nc.sync.dma_start(t_[:], xT[d * P:(d + 1) * P, :])
            xTs.append(t_)

        # q/k head rows [hd, T] bf16 (RoPE applied in place)
        qh = [qk_pool.tile([P, T], BF16, name=f"qh{h}", tag=f"qh{h}")
              for h in range(NHC)]
        kh = [qk_pool.tile([P, T], BF16, name=f"kh{h}", tag=f"kh{h}")
              for h in range(NHC)]

        # v t-tiles [128 keys, 512 cc] bf16
        vhs = [vh_pool.tile([P, CC], BF16, name=f"vh{ti}", tag=f"vh{ti}")
               for ti in range(NT)]

        # attention output yT [hd, h, T] bf16
        yT4 = yT_pool.tile([P, NHC, T], BF16)

        # ------------- helpers --------------------------------------------
        def rope_inplace(dst, half):
            """Apply partial RoPE to dst[0:64, half*HBLK:(half+1)*HBLK]."""
            c0 = half * HBLK
            sl = slice(c0, c0 + HBLK)
            tmp = rope_pool.tile([64, HBLK], BF16, name=f"rp{half}", tag="rp")
            nc.vector.tensor_copy(out=tmp[0:32], in_=dst[32:64, sl])
            nc.vector.tensor_copy(out=tmp[32:64], in_=dst[0:32, sl])
            nc.vector.tensor_mul(out=tmp[:], in0=tmp[:], in1=tbls[:, sl])
            nc.vector.tensor_mul(out=dst[0:64, sl], in0=dst[0:64, sl],
                                 in1=tblc[:, sl])
            nc.vector.tensor_add(out=dst[0:64, sl], in0=dst[0:64, sl],
                                 in1=tmp[:])

        def qk_row_drain(qp, kind, h, half):
            """PSUM [128, HBLK] -> bf16 row half + RoPE."""
            dst = qh[h] if kind == "q" else kh[h]
            c0 = half * HBLK
            nc.scalar.activation(out=dst[:, c0:c0 + HBLK], in_=qp[:],
                                 func=COPY)
            rope_inplace(dst, half)

        # attention unit: one (head, query-block) of ABLK queries
        class AttnUnit:
            def __init__(self, h, b, st_pool, yacc_pool):
                self.h, self.b = h, b
                self.q0 = b * ABLK
                self.st_pool = st_pool
                self.yacc = yacc_pool.tile([P, ABLK], FP32,
                                           name=f"ya{h}_{b}", tag="yacc")
                self.pts = []
                self.s1 = []
                self.s2 = []
                self.s3 = []

            def step(self, tk):
                h, b, q0 = self.h, self.b, self.q0
                st = self.st_pool.tile([P, ABLK], FP32,
                                       name=f"st{h}_{b}_{tk}", tag="st")
                nc.tensor.matmul(st[:], kh[h][:, tk * P:(tk + 1) * P],
                                 qh[h][:, q0:q0 + ABLK], start=True, stop=True)
                pt = pt_pool.tile([P, ABLK], BF16,
                                  name=f"pt{h}_{b}_{tk}", tag="pt")
                nc.scalar.activation(out=pt[:], in_=st[:], func=EXP,
                                     scale=SOFTMAX_SCALE)
                self.pts.append(pt)
                # eager softmax-sum tree (none of it on the PE):
                # pairs on DVE (bf16), quads on Pool (fp32), eighths on DVE
                if tk % 2 == 1:
                    s1 = s1_pool.tile([P, ABLK], BF16,
                                      name=f"s1_{h}_{b}_{tk}", tag="s1")
                    nc.vector.tensor_add(out=s1[:], in0=self.pts[tk - 1][:],
                                         in1=pt[:])
                    self.s1.append(s1)
                if tk % 4 == 3:
                    s2 = s2_pool.tile([P, ABLK], FP32,
                                      name=f"s2_{h}_{b}_{tk}", tag="s2")
                    nc.gpsimd.tensor_add(out=s2[:], in0=self.s1[-2][:],
                                         in1=self.s1[-1][:])
                    self.s2.append(s2)
                if tk % 8 == 7:
                    s3 = s3_pool.tile([P, ABLK], FP32,
                                      name=f"s3_{h}_{b}_{tk}", tag="s3")
                    nc.vector.tensor_add(out=s3[:], in0=self.s2[-2][:],
                                         in1=self.s2[-1][:])
                    self.s3.append(s3)

            def av(self, tk):
                h = self.h
                nc.tensor.matmul(self.yacc[:],
                                 vhs[tk][:, h * HD:(h + 1) * HD],
                                 self.pts[tk][:],
                                 start=(tk == 0), stop=(tk == NT - 1))

            def finish(self):
                h, b = self.h, self.b
                s4 = rs_pool.tile([P, ABLK], FP32, name=f"s4_{h}_{b}", tag="s4")
                nc.gpsimd.tensor_add(out=s4[:], in0=self.s3[0][:],
                                     in1=self.s3[1][:])
                rsum = rs_pool.tile([P, ABLK], FP32, name=f"sr_{h}_{b}",
                                    tag="sr")
                nc.gpsimd.partition_all_reduce(rsum[:], s4[:], channels=P,
                                               reduce_op=ADD)
                rs = rs_pool.tile([P, ABLK], FP32, name=f"rs_{h}_{b}", tag="rs")
                nc.vector.reciprocal(out=rs[:], in_=rsum[:])
                dst = yT4[:, h, self.q0:self.q0 + ABLK]
                nc.vector.tensor_mul(out=dst, in0=self.yacc[:], in1=rs[:])

        # ------------- phase A: x-stream + v + heads 0-1 ------------------
        qkp_pool = tc.alloc_tile_pool(name="qkp", bufs=2, space="PSUM",
                                      side="right")
        with tc.tile_pool(name="vps", bufs=4, space="PSUM",
                          side="right") as vps_pool:
            a_rows = [("k", 0), ("q", 0), ("k", 1), ("q", 1)]
            for w, (kind, h) in enumerate(a_rows):
                qp = [qkp_pool.tile([P, HBLK], FP32, name=f"qp{kind}{h}{hf}",
                                    tag="qkp") for hf in range(2)]
                vt = [vps_pool.tile([P, CC], FP32, name=f"vp{w}_{j}",
                                    tag="vps") for j in range(4)]
                wt = wts[(kind, h)]
                for d in range(ND):
                    for hf in range(2):
                        for c in range(2):
                            c0 = hf * HBLK + c * ABLK
                            nc.tensor.matmul(
                                qp[hf][:, c * ABLK:(c + 1) * ABLK],
                                wt[:, d, :], xTs[d][:, c0:c0 + ABLK],
                                start=(d == 0), stop=(d == ND - 1))
                    for j in range(4):
                        ti = 4 * w + j
                        nc.tensor.matmul(
                            vt[j][:], xTs[d][:, ti * P:(ti + 1) * P],
                            wvs[d][:], start=(d == 0), stop=(d == ND - 1))
                for hf in range(2):
                    qk_row_drain(qp[hf], kind, h, hf)
                for j in range(4):
                    nc.scalar.activation(out=vhs[4 * w + j][:], in_=vt[j][:],
                                         func=COPY)

        # ------------- phases B & C ---------------------------------------
        with (
            tc.tile_pool(name="st", bufs=2, space="PSUM") as st_pool,
            tc.tile_pool(name="yacc", bufs=2, space="PSUM") as yacc_pool,
        ):  # left-side PSUM, coexists with right-side qkp then op
            # phase B: heads 2-3 projections interleaved with attention.
            # Waves 0-1 carry 2 attention units (head 0), waves 2-3 carry 3
            # (head 1 + head 2 blocks 0-1), with the wave's 64 projection
            # chunk matmuls doled out evenly across the attention steps so
            # every PE step stays ahead of the 612ns exp.  A unit's last AV
            # and its sum-tree finish are emitted after the next unit's
            # first S^T so the exp latency at unit boundaries is hidden.
            b_waves = [("k", 2, [(0, 0), (0, 1)]),
                       ("q", 2, [(0, 2), (0, 3)]),
                       ("k", 3, [(1, 0), (1, 1), (2, 0)]),
                       ("q", 3, [(1, 2), (1, 3), (2, 1)])]
            for kind, h, _ in b_waves:
                # just-in-time stationary prefetch (slot from phase A free)
                wts[(kind, h)] = load_wt(kind, h)

            close_prev = None  # emits previous unit's last AV + finish

            def make_close(u):
                def close():
                    u.av(NT - 1)
                    u.finish()
                return close

            for kind, h, units in b_waves:
                wt = wts[(kind, h)]
                # projection chunks, sequential halves (so each qkp slot
                # frees mid-wave, not at the boundary)
                chunks = [(hf, d, c) for hf in range(2) for d in range(ND)
                          for c in range(2)]
                qp = [None, None]
                nsteps = NT * len(units)
                emitted = 0
                step_i = 0
                for u_hb in units:
                    u = AttnUnit(*u_hb, st_pool, yacc_pool)
                    for i in range(NT):
                        u.step(i)
                        if i == 0 and close_prev is not None:
                            close_prev()
                        if i > 0:
                            u.av(i - 1)
                        target = ((step_i + 1) * len(chunks)) // nsteps
                        while emitted < target:
                            hf, d, c = chunks[emitted]
                            if qp[hf] is None:
                                qp[hf] = qkp_pool.tile(
                                    [P, HBLK], FP32,
                                    name=f"qp{kind}{h}{hf}", tag="qkp")
                            c0 = hf * HBLK + c * ABLK
                            nc.tensor.matmul(
                                qp[hf][:, c * ABLK:(c + 1) * ABLK],
                                wt[:, d, :], xTs[d][:, c0:c0 + ABLK],
                                start=(d == 0), stop=(d == ND - 1))
                            emitted += 1
                            if emitted == 32:
                                qk_row_drain(qp[0], kind, h, 0)
                            elif emitted == 64:
                                qk_row_drain(qp[1], kind, h, 1)
                        step_i += 1
                    close_prev = make_close(u)

            qkp_pool.release()
            xT_pool.release()
            wv_pool.release()
            wt_pool.release()
            # (xT/wv/wt are the top of the left SBUF stack, popped LIFO)

            # phase C: attention units (3,0),(2,2),(3,1),(2,3),(3,2),(3,3);
            # oproj matmuls flow from a pending queue (up to 3 per step),
            # with block b's oproj unlocked ~10 steps after head 3 block b
            # closes so the PE never parks on a normalize still in flight.
            wo_pool = tc.alloc_tile_pool(name="wo", bufs=1)
            wot = wo_pool.tile([P, NHC, DIM], BF16)
            nc.gpsimd.dma_start(wot[:], woT[:, :, :])
            ostage_pool = tc.alloc_tile_pool(name="ostage", bufs=2)

            with tc.tile_pool(name="op", bufs=2, space="PSUM",
                              side="right") as op_pool:
                op_state = {}

                def oproj_step(ti, half, hh, c):
                    """Emit matmul (hh, c) of oproj unit (ti, half)."""
                    if hh == 0 and c == 0:
                        op_state[(ti, half)] = op_pool.tile(
                            [P, HBLK], FP32, name=f"op{ti}_{half}", tag="op")
                    op = op_state[(ti, half)]
                    c0 = half * HBLK + c * ABLK
                    nc.tensor.matmul(op[:, c * ABLK:(c + 1) * ABLK],
                                     yT4[:, hh, ti * P:(ti + 1) * P],
                                     wot[:, hh, c0:c0 + ABLK],
                                     start=(hh == 0), stop=(hh == NHC - 1))
                    if hh == NHC - 1 and c == 1:
                        c0 = half * HBLK
                        os_ = ostage_pool.tile([P, HBLK], FP32,
                                               name=f"os{ti}_{half}", tag="os")
                        nc.vector.tensor_copy(out=os_[:], in_=op[:])
                        nc.sync.dma_start(
                            out[ti * P:(ti + 1) * P, c0:c0 + HBLK], os_[:])

                def oproj_block(b):
                    steps = []
                    for j in range(NTB):
                        for hf in range(2):
                            for hh in range(NHC):
                                for c in range(2):
                                    steps.append((b * NTB + j, hf, hh, c))
                    return steps

                c_units = [(3, 0), (2, 2), (3, 1), (2, 3), (3, 2), (3, 3)]
                pending = []
                locked = []  # (unlock_step, op step list)
                gstep = 0
                for h, b in c_units:
                    u = AttnUnit(h, b, st_pool, yacc_pool)
                    for i in range(NT):
                        u.step(i)
                        if i == 0 and close_prev is not None:
                            close_prev()
                        if i > 0:
                            u.av(i - 1)
                        for entry in locked[:]:
                            if entry[0] <= gstep:
                                pending.extend(entry[1])
                                locked.remove(entry)
                        for _ in range(3):
                            if pending:
                                oproj_step(*pending.pop(0))
                        gstep += 1
                    close_prev = make_close(u)
                    if h == 3:
                        locked.append((gstep + 10, oproj_block(b)))
                close_prev()
                close_prev = None
                # tail: remaining oproj work (at least block 3)
                for entry in locked:
                    pending.extend(entry[1])
                for step in pending:
                    oproj_step(*step)

            ostage_pool.release()
            wo_pool.release()

        rs_pool.release()
        s3_pool.release()
        s2_pool.release()
        s1_pool.release()
        pt_pool.release()
        rope_pool.release()
        yT_pool.release()
        vh_pool.release()
        qk_pool.release()
        const_pool.release()

    nc.finalize()
    return nc


def _get_nc():
    if "nc" not in _CACHED:
        _CACHED["nc"] = _build()
    return _CACHED["nc"]


def _in_maps(x, wq, wk, wv, wo):
    import ml_dtypes

    BF = ml_dtypes.bfloat16
    tblc, tbls = _rope_tables()
    in_maps = []
    for core in range(8):
        b, g = divmod(core, 4)
        rows = slice(g * CC, (g + 1) * CC)
        in_maps.append({
            "xT": np.ascontiguousarray(x[b].T).astype(BF),
            "wqT": np.ascontiguousarray(
                wq[rows].reshape(NHC, HD, ND, P).transpose(0, 3, 2, 1)
            ).astype(BF),
            "wkT": np.ascontiguousarray(
                wk[rows].reshape(NHC, HD, ND, P).transpose(0, 3, 2, 1)
            ).astype(BF),
            "wvT": np.ascontiguousarray(wv[rows].T).astype(BF),
            "woT": np.ascontiguousarray(
                wo[:, rows].T.reshape(NHC, P, DIM).transpose(1, 0, 2)
            ).astype(BF),
            "tblc": tblc,
            "tbls": tbls,
        })
    return in_maps


def kernel(x, wq, wk, wv, wo):
    from concourse.bass_utils import run_bass_kernel_spmd

    x = np.asarray(x, dtype=np.float32)
    wq = np.asarray(wq, dtype=np.float32)
    wk = np.asarray(wk, dtype=np.float32)
    wv = np.asarray(wv, dtype=np.float32)
    wo = np.asarray(wo, dtype=np.float32)

    nc = _get_nc()
    res = run_bass_kernel_spmd(nc, _in_maps(x, wq, wk, wv, wo),
                               core_ids=list(range(8)))
    out = np.empty((B, T, DIM), dtype=np.float32)
    for b in range(B):
        out[b] = (res.results[4 * b + 0]["out"] + res.results[4 * b + 1]["out"]
                  + res.results[4 * b + 2]["out"] + res.results[4 * b + 3]["out"])
    return out


# revision 13
# speedup vs baseline: 1.0865x; 1.0628x over previous
"""Multi-head attention (B=2, T=2048, DIM=2048, NH=16, HD=128, partial RoPE)
on 8 Trainium2 NeuronCores.

Sharding (Megatron-style): core c handles batch b = c // 4 and head group
g = c % 4 (heads 4g..4g+3, i.e. 512 of the 2048 q/k/v channels). Each core
computes its heads' attention output and the partial output projection
y_heads @ wo[:, cols].T -> [T, DIM]; the host sums the 4 partials per batch.

Single fused pipeline per core (no DRAM scratch, bf16 datapath, fp32 accum):

  Phase A (x-stream + projections): xT streams in once as bf16 d-chunks; four
  waves each accumulate 4 v t-tiles (PSUM, 1 bank each) + one full q/k head
  row (two [128,1024] halves, 2 banks each) with the d-loop outermost, so the
  PE consumes each arriving x chunk across 8 open PSUM banks.  v tiles drain
  PSUM->SBUF bf16 on Pool; q/k halves drain on ACT (identity copy) and get
  RoPE applied in-place on DVE.  After wave A4: v complete, heads 0-1 ready.

  Phase B: remaining q/k rows (heads 2-3) interleave with attention units for
  heads 0-1.  An attention unit is one (head, 512-query block): per key tile,
  an S^T matmul -> exp on ACT (scale=1/sqrt(hd), no max subtraction; logits
  O(5)) -> bf16 P^T -> AV accumulation into PSUM, with one projection d-step
  between attention steps so the PE never waits on ACT.  Softmax sums never
  touch the PE: an eager pair/quad/eighth add tree alternates DVE (bf16) and
  Pool (fp32), finishing with partition_all_reduce on Pool; reciprocal +
  normalize-mul on DVE write yT bf16.

  Phase C: attention for heads 2-3, with the output projection of query
  block b-1 (yT^T @ woT, 4-head PSUM accumulation) interleaved one matmul
  per attention step; results drain on Pool and DMA out on SP.

  DMA queues: SP carries the xT stream and output stores; Pool carries
  weights/tables (wv chunks interleaved between the early wq/wk stationary
  loads; heads 2-3 stationaries and woT loaded just-in-time).  All matmuls
  run bf16 (full PE rate, fp32 PSUM accumulation).
"""

import math

import numpy as np

B, T, DIM, NH = 2, 2048, 2048, 16
HD = DIM // NH          # 128
P = 128
NHC = 4                 # heads per core
CC = NHC * HD           # 512 channels per core
ND = DIM // P           # 16 d-chunks
NT = T // P             # 16 token tiles
HBLK = 1024             # q/k projection half width
ABLK = 512              # attention query block
NAB = T // ABLK         # 4
NTB = ABLK // P         # t-tiles per attention block (4)
SOFTMAX_SCALE = 1.0 / math.sqrt(HD)

_CACHED = {}


def _rope_tables():
    """tblc [64, T] = cos(invfreq_j * t); tbls [64, T]: rows 0:32 = -sin_j,
    rows 32:64 = +sin_j (swap-term). Used as
      tmp[0:32]  = copy(q[32:64]);  tmp[32:64] = copy(q[0:32])
      tmp[0:64] *= tbls;  q[0:64] *= tblc;  q[0:64] += tmp
    """
    import ml_dtypes

    inv_freq = 1.0 / (10000.0 ** (np.arange(0, HD, 2, dtype=np.float64) / HD))
    t = np.arange(T, dtype=np.float64)
    ang = np.outer(inv_freq, t)              # [64, T]
    tblc = np.cos(ang).astype(ml_dtypes.bfloat16)
    sin = np.sin(ang)
    tbls = np.empty((64, T), dtype=np.float64)
    tbls[0:32] = -sin[0:32]
    tbls[32:64] = sin[32:64]
    return tblc, tbls.astype(ml_dtypes.bfloat16)


def _build():
    import concourse.bacc as bacc
    import concourse.bass_isa as bass_isa
    import concourse.mybir as mybir
    import concourse.tile as tile

    FP32 = mybir.dt.float32
    BF16 = mybir.dt.bfloat16
    EXP = mybir.ActivationFunctionType.Exp
    COPY = mybir.ActivationFunctionType.Copy
    ADD = bass_isa.ReduceOp.add

    nc = bacc.Bacc("TRN2", target_bir_lowering=False, debug=False, num_devices=8)
    xT = nc.declare_dram_parameter("xT", [DIM, T], BF16, isOutput=False)
    wqT = nc.declare_dram_parameter("wqT", [NHC, P, ND, HD], BF16,
                                    isOutput=False)
    wkT = nc.declare_dram_parameter("wkT", [NHC, P, ND, HD], BF16,
                                    isOutput=False)
    wvT = nc.declare_dram_parameter("wvT", [DIM, CC], BF16, isOutput=False)
    woT = nc.declare_dram_parameter("woT", [P, NHC, DIM], BF16, isOutput=False)
    tblcp = nc.declare_dram_parameter("tblc", [64, T], BF16, isOutput=False)
    tblsp = nc.declare_dram_parameter("tbls", [64, T], BF16, isOutput=False)
    out = nc.declare_dram_parameter("out", [T, DIM], FP32, isOutput=True)

    with tile.TileContext(nc) as tc:
        # ------------- SBUF pools (alloc order = reverse release order) ----
        const_pool = tc.alloc_tile_pool(name="const", bufs=1)
        qk_pool = tc.alloc_tile_pool(name="qk", bufs=1)
        vh_pool = tc.alloc_tile_pool(name="vh", bufs=1)
        yT_pool = tc.alloc_tile_pool(name="yT", bufs=1)
        rope_pool = tc.alloc_tile_pool(name="rope", bufs=1)
        pt_pool = tc.alloc_tile_pool(name="pt", bufs=4)
        s1_pool = tc.alloc_tile_pool(name="s1", bufs=4)
        s2_pool = tc.alloc_tile_pool(name="s2", bufs=3)
        s3_pool = tc.alloc_tile_pool(name="s3", bufs=3)
        rs_pool = tc.alloc_tile_pool(name="rs", bufs=2)
        # phase-scoped pools (released end of phase B, LIFO)
        wt_pool = tc.alloc_tile_pool(name="wt", bufs=4)
        wv_pool = tc.alloc_tile_pool(name="wv", bufs=1)
        xT_pool = tc.alloc_tile_pool(name="xT", bufs=1)

        def load_wt(kind, h):
            src = wqT if kind == "q" else wkT
            wt = wt_pool.tile([P, ND, HD], BF16, name=f"wt{kind}{h}", tag="wt")
            nc.gpsimd.dma_start(wt[:], src[h])
            return wt

        # wv moving tiles (one per d-chunk), interleaved with the early wt
        # loads on the Pool queue so wv0 lands ~2us in
        wvs = [None] * ND

        def load_wv(lo, hi):
            for d in range(lo, hi):
                w_ = wv_pool.tile([P, CC], BF16, name=f"wv{d}", tag=f"wv{d}")
                nc.gpsimd.dma_start(w_[:], wvT[d * P:(d + 1) * P, :])
                wvs[d] = w_

        wts = {}
        wts[("k", 0)] = load_wt("k", 0)
        load_wv(0, 4)
        wts[("q", 0)] = load_wt("q", 0)
        load_wv(4, 8)
        wts[("k", 1)] = load_wt("k", 1)
        load_wv(8, 12)
        wts[("q", 1)] = load_wt("q", 1)
        load_wv(12, 16)
        # RoPE tables load after the critical-path weights
        tblc = const_pool.tile([64, T], BF16)
        nc.gpsimd.dma_start(tblc[:], tblcp[:, :])
        tbls = const_pool.tile([64, T], BF16)
        nc.gpsimd.dma_start(tbls[:], tblsp[:, :])

        # xT resident d-chunk tiles, streamed on SP
        xTs = []
        for d in range(ND):
            t_ = xT_pool.tile([P, T], BF16, name=f"xt{d}", tag=f"xt{d}")
            nc.sync.dma_start(t_[:], xT[d * P:(d + 1) * P, :])
            xTs.append(t_)

        # q/k head rows [hd, T] bf16 (RoPE applied in place)
        qh = [qk_pool.tile([P, T], BF16, name=f"qh{h}", tag=f"qh{h}")
              for h in range(NHC)]
        kh = [qk_pool.tile([P, T], BF16, name=f"kh{h}", tag=f"kh{h}")
              for h in range(NHC)]

        # v t-tiles [128 keys, 512 cc] bf16
        vhs = [vh_pool.tile([P, CC], BF16, name=f"vh{ti}", tag=f"vh{ti}")
               for ti in range(NT)]

        # attention output yT [hd, h, T] bf16
        yT4 = yT_pool.tile([P, NHC, T], BF16)

        # ------------- helpers --------------------------------------------
        def rope_inplace(dst, half):
            """Apply partial RoPE to dst[0:64, half*HBLK:(half+1)*HBLK]."""
            c0 = half * HBLK
            sl = slice(c0, c0 + HBLK)
            tmp = rope_pool.tile([64, HBLK], BF16, name=f"rp{half}", tag="rp")
            nc.vector.tensor_copy(out=tmp[0:32], in_=dst[32:64, sl])
            nc.vector.tensor_copy(out=tmp[32:64], in_=dst[0:32, sl])
            nc.vector.tensor_mul(out=tmp[:], in0=tmp[:], in1=tbls[:, sl])
            nc.vector.tensor_mul(out=dst[0:64, sl], in0=dst[0:64, sl],
                                 in1=tblc[:, sl])
            nc.vector.tensor_add(out=dst[0:64, sl], in0=dst[0:64, sl],
                                 in1=tmp[:])

        def qk_row_drain(qp, kind, h, half):
            """PSUM [128, HBLK] -> bf16 row half + RoPE."""
            dst = qh[h] if kind == "q" else kh[h]
            c0 = half * HBLK
            nc.scalar.activation(out=dst[:, c0:c0 + HBLK], in_=qp[:],
                                 func=COPY)
            rope_inplace(dst, half)

        # attention unit: one (head, query-block) of ABLK queries
        class AttnUnit:
            def __init__(self, h, b, st_pool, yacc_pool):
                self.h, self.b = h, b
                self.q0 = b * ABLK
                self.st_pool = st_pool
                self.yacc = yacc_pool.tile([P, ABLK], FP32,
                                           name=f"ya{h}_{b}", tag="yacc")
                self.pts = []
                self.s1 = []
                self.s2 = []
                self.s3 = []

            def step(self, tk):
                h, b, q0 = self.h, self.b, self.q0
                st = self.st_pool.tile([P, ABLK], FP32,
                                       name=f"st{h}_{b}_{tk}", tag="st")
                nc.tensor.matmul(st[:], kh[h][:, tk * P:(tk + 1) * P],
                                 qh[h][:, q0:q0 + ABLK], start=True, stop=True)
                pt = pt_pool.tile([P, ABLK], BF16,
                                  name=f"pt{h}_{b}_{tk}", tag="pt")
                nc.scalar.activation(out=pt[:], in_=st[:], func=EXP,
                                     scale=SOFTMAX_SCALE)
                self.pts.append(pt)
                # eager softmax-sum tree (none of it on the PE):
                # pairs on DVE (bf16), quads on Pool (fp32), eighths on DVE
                if tk % 2 == 1:
                    s1 = s1_pool.tile([P, ABLK], BF16,
                                      name=f"s1_{h}_{b}_{tk}", tag="s1")
                    nc.vector.tensor_add(out=s1[:], in0=self.pts[tk - 1][:],
                                         in1=pt[:])
                    self.s1.append(s1)
                if tk % 4 == 3:
                    # last quad-add on DVE (normalize critical path);
                    # eager ones on Pool
                    eng = nc.vector if tk == NT - 1 else nc.gpsimd
                    s2 = s2_pool.tile([P, ABLK], FP32,
                                      name=f"s2_{h}_{b}_{tk}", tag="s2")
                    eng.tensor_add(out=s2[:], in0=self.s1[-2][:],
                                   in1=self.s1[-1][:])
                    self.s2.append(s2)
                if tk % 8 == 7:
                    eng = nc.vector if tk == NT - 1 else nc.gpsimd
                    s3 = s3_pool.tile([P, ABLK], FP32,
                                      name=f"s3_{h}_{b}_{tk}", tag="s3")
                    eng.tensor_add(out=s3[:], in0=self.s2[-2][:],
                                   in1=self.s2[-1][:])
                    self.s3.append(s3)
                    if tk == NT // 2 - 1:
                        # eager partition-reduce of the first key half
                        ar = rs_pool.tile([P, ABLK], FP32,
                                          name=f"ar0_{h}_{b}", tag="ar0")
                        nc.gpsimd.partition_all_reduce(ar[:], s3[:],
                                                       channels=P,
                                                       reduce_op=ADD)
                        self.ar0 = ar

            def av(self, tk):
                h = self.h
                nc.tensor.matmul(self.yacc[:],
                                 vhs[tk][:, h * HD:(h + 1) * HD],
                                 self.pts[tk][:],
                                 start=(tk == 0), stop=(tk == NT - 1))

            def finish(self):
                h, b = self.h, self.b
                ar1 = rs_pool.tile([P, ABLK], FP32, name=f"ar1_{h}_{b}",
                                   tag="ar1")
                nc.gpsimd.partition_all_reduce(ar1[:], self.s3[1][:],
                                               channels=P, reduce_op=ADD)
                rsum = rs_pool.tile([P, ABLK], FP32, name=f"sr_{h}_{b}",
                                    tag="sr")
                nc.vector.tensor_add(out=rsum[:], in0=self.ar0[:],
                                     in1=ar1[:])
                rs = rs_pool.tile([P, ABLK], FP32, name=f"rs_{h}_{b}", tag="rs")
                nc.vector.reciprocal(out=rs[:], in_=rsum[:])
                dst = yT4[:, h, self.q0:self.q0 + ABLK]
                nc.vector.tensor_mul(out=dst, in0=self.yacc[:], in1=rs[:])

        # ------------- phase A: x-stream + v + heads 0-1 ------------------
        qkp_pool = tc.alloc_tile_pool(name="qkp", bufs=2, space="PSUM",
                                      side="right")
        with tc.tile_pool(name="vps", bufs=4, space="PSUM",
                          side="right") as vps_pool:
            a_rows = [("k", 0), ("q", 0), ("k", 1), ("q", 1)]
            for w, (kind, h) in enumerate(a_rows):
                qp = [qkp_pool.tile([P, HBLK], FP32, name=f"qp{kind}{h}{hf}",
                                    tag="qkp") for hf in range(2)]
                vt = [vps_pool.tile([P, CC], FP32, name=f"vp{w}_{j}",
                                    tag="vps") for j in range(4)]
                wt = wts[(kind, h)]
                for d in range(ND):
                    for hf in range(2):
                        for c in range(2):
                            c0 = hf * HBLK + c * ABLK
                            nc.tensor.matmul(
                                qp[hf][:, c * ABLK:(c + 1) * ABLK],
                                wt[:, d, :], xTs[d][:, c0:c0 + ABLK],
                                start=(d == 0), stop=(d == ND - 1))
                    for j in range(4):
                        ti = 4 * w + j
                        nc.tensor.matmul(
                            vt[j][:], xTs[d][:, ti * P:(ti + 1) * P],
                            wvs[d][:], start=(d == 0), stop=(d == ND - 1))
                for hf in range(2):
                    qk_row_drain(qp[hf], kind, h, hf)
                for j in range(4):
                    nc.scalar.activation(out=vhs[4 * w + j][:], in_=vt[j][:],
                                         func=COPY)

        # ------------- phases B & C ---------------------------------------
        with (
            tc.tile_pool(name="st", bufs=2, space="PSUM") as st_pool,
            tc.tile_pool(name="yacc", bufs=2, space="PSUM") as yacc_pool,
        ):  # left-side PSUM, coexists with right-side qkp then op
            # phase B: heads 2-3 projections interleaved with attention.
            # Waves 0-1 carry 2 attention units (head 0), waves 2-3 carry 3
            # (head 1 + head 2 blocks 0-1), with the wave's 64 projection
            # chunk matmuls doled out evenly across the attention steps so
            # every PE step stays ahead of the 612ns exp.  A unit's last AV
            # and its sum-tree finish are emitted after the next unit's
            # first S^T so the exp latency at unit boundaries is hidden.
            b_waves = [("k", 2, [(0, 0), (0, 1)]),
                       ("q", 2, [(0, 2), (0, 3)]),
                       ("k", 3, [(1, 0), (1, 1), (2, 0)]),
                       ("q", 3, [(1, 2), (1, 3), (2, 1)])]
            for kind, h, _ in b_waves:
                # just-in-time stationary prefetch (slot from phase A free)
                wts[(kind, h)] = load_wt(kind, h)

            close_prev = None  # emits previous unit's last AV + finish

            def make_close(u):
                def close():
                    u.av(NT - 1)
                    u.finish()
                return close

            for kind, h, units in b_waves:
                wt = wts[(kind, h)]
                # projection chunks, sequential halves (so each qkp slot
                # frees mid-wave, not at the boundary)
                chunks = [(hf, d, c) for hf in range(2) for d in range(ND)
                          for c in range(2)]
                qp = [None, None]
                nsteps = NT * len(units)
                emitted = 0
                step_i = 0
                for u_hb in units:
                    u = AttnUnit(*u_hb, st_pool, yacc_pool)
                    for i in range(NT):
                        u.step(i)
                        if i == 0 and close_prev is not None:
                            close_prev()
                        if i > 0:
                            u.av(i - 1)
                        target = ((step_i + 1) * len(chunks)) // nsteps
                        while emitted < target:
                            hf, d, c = chunks[emitted]
                            if qp[hf] is None:
                                qp[hf] = qkp_pool.tile(
                                    [P, HBLK], FP32,
                                    name=f"qp{kind}{h}{hf}", tag="qkp")
                            c0 = hf * HBLK + c * ABLK
                            nc.tensor.matmul(
                                qp[hf][:, c * ABLK:(c + 1) * ABLK],
                                wt[:, d, :], xTs[d][:, c0:c0 + ABLK],
                                start=(d == 0), stop=(d == ND - 1))
                            emitted += 1
                            if emitted == 32:
                                qk_row_drain(qp[0], kind, h, 0)
                            elif emitted == 64:
                                qk_row_drain(qp[1], kind, h, 1)
                        step_i += 1
                    close_prev = make_close(u)

            qkp_pool.release()
            xT_pool.release()
            wv_pool.release()
            wt_pool.release()
            # (xT/wv/wt are the top of the left SBUF stack, popped LIFO)

            # phase C: attention units (3,0),(2,2),(3,1),(2,3),(3,2),(3,3);
            # oproj matmuls flow from a pending queue (up to 3 per step),
            # with block b's oproj unlocked ~10 steps after head 3 block b
            # closes so the PE never parks on a normalize still in flight.
            wo_pool = tc.alloc_tile_pool(name="wo", bufs=1)
            wot = wo_pool.tile([P, NHC, DIM], BF16)
            nc.gpsimd.dma_start(wot[:], woT[:, :, :])
            ostage_pool = tc.alloc_tile_pool(name="ostage", bufs=4)

            with tc.tile_pool(name="op", bufs=2, space="PSUM",
                              side="right") as op_pool:
                op_state = {}

                def oproj_step(ti, half, hh, c):
                    """Emit matmul (hh, c) of oproj unit (ti, half)."""
                    if hh == 0 and c == 0:
                        op_state[(ti, half)] = op_pool.tile(
                            [P, HBLK], FP32, name=f"op{ti}_{half}", tag="op")
                    op = op_state[(ti, half)]
                    c0 = half * HBLK + c * ABLK
                    nc.tensor.matmul(op[:, c * ABLK:(c + 1) * ABLK],
                                     yT4[:, hh, ti * P:(ti + 1) * P],
                                     wot[:, hh, c0:c0 + ABLK],
                                     start=(hh == 0), stop=(hh == NHC - 1))
                    if hh == NHC - 1 and c == 1:
                        c0 = half * HBLK
                        os_ = ostage_pool.tile([P, HBLK], FP32,
                                               name=f"os{ti}_{half}", tag="os")
                        nc.vector.tensor_copy(out=os_[:], in_=op[:])
                        eng = nc.sync if (ti + half) % 2 == 0 else nc.gpsimd
                        eng.dma_start(
                            out[ti * P:(ti + 1) * P, c0:c0 + HBLK], os_[:])

                def oproj_block(b):
                    steps = []
                    for j in range(NTB):
                        for hf in range(2):
                            for hh in range(NHC):
                                for c in range(2):
                                    steps.append((b * NTB + j, hf, hh, c))
                    return steps

                c_units = [(3, 0), (2, 2), (3, 1), (2, 3), (3, 2), (3, 3)]
                pending = []
                locked = []  # (unlock_step, op step list)
                gstep = 0
                for h, b in c_units:
                    u = AttnUnit(h, b, st_pool, yacc_pool)
                    for i in range(NT):
                        u.step(i)
                        if i == 0 and close_prev is not None:
                            close_prev()
                        if i > 0:
                            u.av(i - 1)
                        for entry in locked[:]:
                            if entry[0] <= gstep:
                                pending.extend(entry[1])
                                locked.remove(entry)
                        for _ in range(3):
                            if pending:
                                oproj_step(*pending.pop(0))
                        gstep += 1
                    close_prev = make_close(u)
                    if h == 3:
                        locked.append((gstep + 10, oproj_block(b)))
                close_prev()
                close_prev = None
                # tail: remaining oproj work (at least block 3)
                for entry in locked:
                    pending.extend(entry[1])
                for step in pending:
                    oproj_step(*step)

            ostage_pool.release()
            wo_pool.release()

        rs_pool.release()
        s3_pool.release()
        s2_pool.release()
        s1_pool.release()
        pt_pool.release()
        rope_pool.release()
        yT_pool.release()
        vh_pool.release()
        qk_pool.release()
        const_pool.release()

    nc.finalize()
    return nc


def _get_nc():
    if "nc" not in _CACHED:
        _CACHED["nc"] = _build()
    return _CACHED["nc"]


def _in_maps(x, wq, wk, wv, wo):
    import ml_dtypes

    BF = ml_dtypes.bfloat16
    tblc, tbls = _rope_tables()
    in_maps = []
    for core in range(8):
        b, g = divmod(core, 4)
        rows = slice(g * CC, (g + 1) * CC)
        in_maps.append({
            "xT": np.ascontiguousarray(x[b].T).astype(BF),
            "wqT": np.ascontiguousarray(
                wq[rows].reshape(NHC, HD, ND, P).transpose(0, 3, 2, 1)
            ).astype(BF),
            "wkT": np.ascontiguousarray(
                wk[rows].reshape(NHC, HD, ND, P).transpose(0, 3, 2, 1)
            ).astype(BF),
            "wvT": np.ascontiguousarray(wv[rows].T).astype(BF),
            "woT": np.ascontiguousarray(
                wo[:, rows].T.reshape(NHC, P, DIM).transpose(1, 0, 2)
            ).astype(BF),
            "tblc": tblc,
            "tbls": tbls,
        })
    return in_maps


def kernel(x, wq, wk, wv, wo):
    from concourse.bass_utils import run_bass_kernel_spmd

    x = np.asarray(x, dtype=np.float32)
    wq = np.asarray(wq, dtype=np.float32)
    wk = np.asarray(wk, dtype=np.float32)
    wv = np.asarray(wv, dtype=np.float32)
    wo = np.asarray(wo, dtype=np.float32)

    nc = _get_nc()
    res = run_bass_kernel_spmd(nc, _in_maps(x, wq, wk, wv, wo),
                               core_ids=list(range(8)))
    out = np.empty((B, T, DIM), dtype=np.float32)
    for b in range(B):
        out[b] = (res.results[4 * b + 0]["out"] + res.results[4 * b + 1]["out"]
                  + res.results[4 * b + 2]["out"] + res.results[4 * b + 3]["out"])
    return out


# revision 15
# speedup vs baseline: 1.0899x; 1.0032x over previous
"""Multi-head attention (B=2, T=2048, DIM=2048, NH=16, HD=128, partial RoPE)
on 8 Trainium2 NeuronCores.

Sharding (Megatron-style): core c handles batch b = c // 4 and head group
g = c % 4 (heads 4g..4g+3, i.e. 512 of the 2048 q/k/v channels). Each core
computes its heads' attention output and the partial output projection
y_heads @ wo[:, cols].T -> [T, DIM]; the host sums the 4 partials per batch.

Single fused pipeline per core (no DRAM scratch, bf16 datapath, fp32 accum):

  Phase A (x-stream + projections): xT streams in once as bf16 d-chunks; four
  waves each accumulate 4 v t-tiles (PSUM, 1 bank each) + one full q/k head
  row (two [128,1024] halves, 2 banks each) with the d-loop outermost, so the
  PE consumes each arriving x chunk across 8 open PSUM banks.  v tiles drain
  PSUM->SBUF bf16 on Pool; q/k halves drain on ACT (identity copy) and get
  RoPE applied in-place on DVE.  After wave A4: v complete, heads 0-1 ready.

  Phase B: remaining q/k rows (heads 2-3) interleave with attention units for
  heads 0-1.  An attention unit is one (head, 512-query block): per key tile,
  an S^T matmul -> exp on ACT (scale=1/sqrt(hd), no max subtraction; logits
  O(5)) -> bf16 P^T -> AV accumulation into PSUM, with one projection d-step
  between attention steps so the PE never waits on ACT.  Softmax sums never
  touch the PE: an eager pair/quad/eighth add tree alternates DVE (bf16) and
  Pool (fp32), finishing with partition_all_reduce on Pool; reciprocal +
  normalize-mul on DVE write yT bf16.

  Phase C: attention for heads 2-3, with the output projection of query
  block b-1 (yT^T @ woT, 4-head PSUM accumulation) interleaved one matmul
  per attention step; results drain on Pool and DMA out on SP.

  DMA queues: SP carries the xT stream and output stores; Pool carries
  weights/tables (wv chunks interleaved between the early wq/wk stationary
  loads; heads 2-3 stationaries and woT loaded just-in-time).  All matmuls
  run bf16 (full PE rate, fp32 PSUM accumulation).
"""

import math

import numpy as np

B, T, DIM, NH = 2, 2048, 2048, 16
HD = DIM // NH          # 128
P = 128
NHC = 4                 # heads per core
CC = NHC * HD           # 512 channels per core
ND = DIM // P           # 16 d-chunks
NT = T // P             # 16 token tiles
HBLK = 1024             # q/k projection half width
ABLK = 512              # attention query block
NAB = T // ABLK         # 4
NTB = ABLK // P         # t-tiles per attention block (4)
SOFTMAX_SCALE = 1.0 / math.sqrt(HD)

_CACHED = {}


def _rope_tables():
    """tblc [64, T] = cos(invfreq_j * t); tbls [64, T]: rows 0:32 = -sin_j,
    rows 32:64 = +sin_j (swap-term). Used as
      tmp[0:32]  = copy(q[32:64]);  tmp[32:64] = copy(q[0:32])
      tmp[0:64] *= tbls;  q[0:64] *= tblc;  q[0:64] += tmp
    """
    import ml_dtypes

    inv_freq = 1.0 / (10000.0 ** (np.arange(0, HD, 2, dtype=np.float64) / HD))
    t = np.arange(T, dtype=np.float64)
    ang = np.outer(inv_freq, t)              # [64, T]
    tblc = np.cos(ang).astype(ml_dtypes.bfloat16)
    sin = np.sin(ang)
    tbls = np.empty((64, T), dtype=np.float64)
    tbls[0:32] = -sin[0:32]
    tbls[32:64] = sin[32:64]
    return tblc, tbls.astype(ml_dtypes.bfloat16)


def _build():
    import concourse.bacc as bacc
    import concourse.bass_isa as bass_isa
    import concourse.mybir as mybir
    import concourse.tile as tile

    FP32 = mybir.dt.float32
    BF16 = mybir.dt.bfloat16
    EXP = mybir.ActivationFunctionType.Exp
    COPY = mybir.ActivationFunctionType.Copy
    ADD = bass_isa.ReduceOp.add

    nc = bacc.Bacc("TRN2", target_bir_lowering=False, debug=False, num_devices=8)
    xT = nc.declare_dram_parameter("xT", [DIM, T], BF16, isOutput=False)
    wqT = nc.declare_dram_parameter("wqT", [NHC, P, ND, HD], BF16,
                                    isOutput=False)
    wkT = nc.declare_dram_parameter("wkT", [NHC, P, ND, HD], BF16,
                                    isOutput=False)
    wvT = nc.declare_dram_parameter("wvT", [DIM, CC], BF16, isOutput=False)
    woT = nc.declare_dram_parameter("woT", [P, NHC, DIM], BF16, isOutput=False)
    tblcp = nc.declare_dram_parameter("tblc", [64, T], BF16, isOutput=False)
    tblsp = nc.declare_dram_parameter("tbls", [64, T], BF16, isOutput=False)
    out = nc.declare_dram_parameter("out", [T, DIM], FP32, isOutput=True)

    with tile.TileContext(nc) as tc:
        # ------------- SBUF pools (alloc order = reverse release order) ----
        const_pool = tc.alloc_tile_pool(name="const", bufs=1)
        qk_pool = tc.alloc_tile_pool(name="qk", bufs=1)
        vh_pool = tc.alloc_tile_pool(name="vh", bufs=1)
        yT_pool = tc.alloc_tile_pool(name="yT", bufs=1)
        rope_pool = tc.alloc_tile_pool(name="rope", bufs=1)
        pt_pool = tc.alloc_tile_pool(name="pt", bufs=4)
        s1_pool = tc.alloc_tile_pool(name="s1", bufs=4)
        s2_pool = tc.alloc_tile_pool(name="s2", bufs=3)
        s3_pool = tc.alloc_tile_pool(name="s3", bufs=3)
        rs_pool = tc.alloc_tile_pool(name="rs", bufs=2)
        # phase-scoped pools (released end of phase B, LIFO)
        wt_pool = tc.alloc_tile_pool(name="wt", bufs=4)
        wv_pool = tc.alloc_tile_pool(name="wv", bufs=1)
        xT_pool = tc.alloc_tile_pool(name="xT", bufs=1)

        def load_wt(kind, h):
            src = wqT if kind == "q" else wkT
            wt = wt_pool.tile([P, ND, HD], BF16, name=f"wt{kind}{h}", tag="wt")
            nc.gpsimd.dma_start(wt[:], src[h])
            return wt

        # wv moving tiles (one per d-chunk), interleaved with the early wt
        # loads on the Pool queue so wv0 lands ~2us in
        wvs = [None] * ND

        def load_wv(lo, hi):
            for d in range(lo, hi):
                w_ = wv_pool.tile([P, CC], BF16, name=f"wv{d}", tag=f"wv{d}")
                nc.gpsimd.dma_start(w_[:], wvT[d * P:(d + 1) * P, :])
                wvs[d] = w_

        wts = {}
        wts[("k", 0)] = load_wt("k", 0)
        load_wv(0, 16)
        wts[("q", 0)] = load_wt("q", 0)
        wts[("k", 1)] = load_wt("k", 1)
        wts[("q", 1)] = load_wt("q", 1)
        # RoPE tables load after the critical-path weights
        tblc = const_pool.tile([64, T], BF16)
        nc.gpsimd.dma_start(tblc[:], tblcp[:, :])
        tbls = const_pool.tile([64, T], BF16)
        nc.gpsimd.dma_start(tbls[:], tblsp[:, :])

        # xT resident d-chunk tiles, streamed on SP
        xTs = []
        for d in range(ND):
            t_ = xT_pool.tile([P, T], BF16, name=f"xt{d}", tag=f"xt{d}")
            nc.sync.dma_start(t_[:], xT[d * P:(d + 1) * P, :])
            xTs.append(t_)

        # q/k head rows [hd, T] bf16 (RoPE applied in place)
        qh = [qk_pool.tile([P, T], BF16, name=f"qh{h}", tag=f"qh{h}")
              for h in range(NHC)]
        kh = [qk_pool.tile([P, T], BF16, name=f"kh{h}", tag=f"kh{h}")
              for h in range(NHC)]

        # v t-tiles [128 keys, 512 cc] bf16
        vhs = [vh_pool.tile([P, CC], BF16, name=f"vh{ti}", tag=f"vh{ti}")
               for ti in range(NT)]

        # attention output yT [hd, h, T] bf16
        yT4 = yT_pool.tile([P, NHC, T], BF16)

        # ------------- helpers --------------------------------------------
        def rope_inplace(dst, half):
            """Apply partial RoPE to dst[0:64, half*HBLK:(half+1)*HBLK]."""
            c0 = half * HBLK
            sl = slice(c0, c0 + HBLK)
            tmp = rope_pool.tile([64, HBLK], BF16, name=f"rp{half}", tag="rp")
            nc.vector.tensor_copy(out=tmp[0:32], in_=dst[32:64, sl])
            nc.vector.tensor_copy(out=tmp[32:64], in_=dst[0:32, sl])
            nc.vector.tensor_mul(out=tmp[:], in0=tmp[:], in1=tbls[:, sl])
            nc.vector.tensor_mul(out=dst[0:64, sl], in0=dst[0:64, sl],
                                 in1=tblc[:, sl])
            nc.vector.tensor_add(out=dst[0:64, sl], in0=dst[0:64, sl],
                                 in1=tmp[:])

        def qk_row_drain(qp, kind, h, half):
            """PSUM [128, HBLK] -> bf16 row half + RoPE."""
            dst = qh[h] if kind == "q" else kh[h]
            c0 = half * HBLK
            nc.scalar.activation(out=dst[:, c0:c0 + HBLK], in_=qp[:],
                                 func=COPY)
            rope_inplace(dst, half)

        # attention unit: one (head, query-block) of ABLK queries
        class AttnUnit:
            def __init__(self, h, b, st_pool, yacc_pool):
                self.h, self.b = h, b
                self.q0 = b * ABLK
                self.st_pool = st_pool
                self.yacc = yacc_pool.tile([P, ABLK], FP32,
                                           name=f"ya{h}_{b}", tag="yacc")
                self.pts = []
                self.s1 = []
                self.s2 = []
                self.s3 = []

            def step(self, tk):
                h, b, q0 = self.h, self.b, self.q0
                st = self.st_pool.tile([P, ABLK], FP32,
                                       name=f"st{h}_{b}_{tk}", tag="st")
                nc.tensor.matmul(st[:], kh[h][:, tk * P:(tk + 1) * P],
                                 qh[h][:, q0:q0 + ABLK], start=True, stop=True)
                pt = pt_pool.tile([P, ABLK], BF16,
                                  name=f"pt{h}_{b}_{tk}", tag="pt")
                nc.scalar.activation(out=pt[:], in_=st[:], func=EXP,
                                     scale=SOFTMAX_SCALE)
                self.pts.append(pt)
                # eager softmax-sum tree (none of it on the PE):
                # pairs on DVE (bf16), quads on Pool (fp32), eighths on DVE
                if tk % 2 == 1:
                    s1 = s1_pool.tile([P, ABLK], BF16,
                                      name=f"s1_{h}_{b}_{tk}", tag="s1")
                    nc.vector.tensor_add(out=s1[:], in0=self.pts[tk - 1][:],
                                         in1=pt[:])
                    self.s1.append(s1)
                if tk % 4 == 3:
                    # last quad-add on DVE (normalize critical path);
                    # eager ones on Pool
                    eng = nc.vector if tk == NT - 1 else nc.gpsimd
                    s2 = s2_pool.tile([P, ABLK], FP32,
                                      name=f"s2_{h}_{b}_{tk}", tag="s2")
                    eng.tensor_add(out=s2[:], in0=self.s1[-2][:],
                                   in1=self.s1[-1][:])
                    self.s2.append(s2)
                if tk % 8 == 7:
                    eng = nc.vector if tk == NT - 1 else nc.gpsimd
                    s3 = s3_pool.tile([P, ABLK], FP32,
                                      name=f"s3_{h}_{b}_{tk}", tag="s3")
                    eng.tensor_add(out=s3[:], in0=self.s2[-2][:],
                                   in1=self.s2[-1][:])
                    self.s3.append(s3)
                    if tk == NT // 2 - 1:
                        # eager partition-reduce of the first key half
                        ar = rs_pool.tile([P, ABLK], FP32,
                                          name=f"ar0_{h}_{b}", tag="ar0")
                        nc.gpsimd.partition_all_reduce(ar[:], s3[:],
                                                       channels=P,
                                                       reduce_op=ADD)
                        self.ar0 = ar

            def av(self, tk):
                h = self.h
                nc.tensor.matmul(self.yacc[:],
                                 vhs[tk][:, h * HD:(h + 1) * HD],
                                 self.pts[tk][:],
                                 start=(tk == 0), stop=(tk == NT - 1))

            def finish(self):
                h, b = self.h, self.b
                ar1 = rs_pool.tile([P, ABLK], FP32, name=f"ar1_{h}_{b}",
                                   tag="ar1")
                nc.gpsimd.partition_all_reduce(ar1[:], self.s3[1][:],
                                               channels=P, reduce_op=ADD)
                rsum = rs_pool.tile([P, ABLK], FP32, name=f"sr_{h}_{b}",
                                    tag="sr")
                nc.vector.tensor_add(out=rsum[:], in0=self.ar0[:],
                                     in1=ar1[:])
                rs = rs_pool.tile([P, ABLK], FP32, name=f"rs_{h}_{b}", tag="rs")
                nc.vector.reciprocal(out=rs[:], in_=rsum[:])
                dst = yT4[:, h, self.q0:self.q0 + ABLK]
                nc.vector.tensor_mul(out=dst, in0=self.yacc[:], in1=rs[:])

        # ------------- phase A: x-stream + v + heads 0-1 ------------------
        qkp_pool = tc.alloc_tile_pool(name="qkp", bufs=2, space="PSUM",
                                      side="right")
        with tc.tile_pool(name="vps", bufs=4, space="PSUM",
                          side="right") as vps_pool:
            a_rows = [("k", 0), ("q", 0), ("k", 1), ("q", 1)]
            for w, (kind, h) in enumerate(a_rows):
                qp = [qkp_pool.tile([P, HBLK], FP32, name=f"qp{kind}{h}{hf}",
                                    tag="qkp") for hf in range(2)]
                vt = [vps_pool.tile([P, CC], FP32, name=f"vp{w}_{j}",
                                    tag="vps") for j in range(4)]
                wt = wts[(kind, h)]
                for d in range(ND):
                    for hf in range(2):
                        for c in range(2):
                            c0 = hf * HBLK + c * ABLK
                            nc.tensor.matmul(
                                qp[hf][:, c * ABLK:(c + 1) * ABLK],
                                wt[:, d, :], xTs[d][:, c0:c0 + ABLK],
                                start=(d == 0), stop=(d == ND - 1))
                    for j in range(4):
                        ti = 4 * w + j
                        nc.tensor.matmul(
                            vt[j][:], xTs[d][:, ti * P:(ti + 1) * P],
                            wvs[d][:], start=(d == 0), stop=(d == ND - 1))
                for hf in range(2):
                    qk_row_drain(qp[hf], kind, h, hf)
                for j in range(4):
                    nc.scalar.activation(out=vhs[4 * w + j][:], in_=vt[j][:],
                                         func=COPY)

        # ------------- phases B & C ---------------------------------------
        with (
            tc.tile_pool(name="st", bufs=2, space="PSUM") as st_pool,
            tc.tile_pool(name="yacc", bufs=2, space="PSUM") as yacc_pool,
        ):  # left-side PSUM, coexists with right-side qkp then op
            # phase B: remaining projection rows (k2,q2,k3,q3) doled
            # uniformly (1.6 chunk matmuls per attention step) under a flat
            # run of 10 attention units; head-2 blocks 0-1 and head-3
            # block 0 join as soon as their q/k rows have drained.  A
            # unit's last AV + sum-tree finish are emitted after the next
            # unit's first S^T so exp latency at boundaries stays hidden.
            b_rows = [("k", 2), ("q", 2), ("k", 3), ("q", 3)]
            for kind, h in b_rows:
                # just-in-time stationary prefetch (slot from phase A free)
                wts[(kind, h)] = load_wt(kind, h)
            b_units = [(0, 0), (0, 1), (0, 2), (0, 3), (2, 0),
                       (1, 0), (1, 1), (2, 1), (1, 2), (3, 0)]
            chunks = []  # (row_idx, hf, d, c) in accumulation order
            for r in range(len(b_rows)):
                for hf in range(2):
                    for d in range(ND):
                        for c in range(2):
                            chunks.append((r, hf, d, c))
            qp = {}
            nsteps = NT * len(b_units)
            emitted = 0
            step_i = 0

            close_prev = None  # emits previous unit's last AV + finish

            def make_close(u):
                def close():
                    u.av(NT - 1)
                    u.finish()
                return close

            for u_hb in b_units:
                u = AttnUnit(*u_hb, st_pool, yacc_pool)
                for i in range(NT):
                    u.step(i)
                    if i == 0 and close_prev is not None:
                        close_prev()
                    if i > 0:
                        u.av(i - 1)
                    target = ((step_i + 1) * len(chunks)) // nsteps
                    while emitted < target:
                        r, hf, d, c = chunks[emitted]
                        kind, h = b_rows[r]
                        if (r, hf) not in qp:
                            qp[(r, hf)] = qkp_pool.tile(
                                [P, HBLK], FP32,
                                name=f"qp{kind}{h}{hf}", tag="qkp")
                        c0 = hf * HBLK + c * ABLK
                        nc.tensor.matmul(
                            qp[(r, hf)][:, c * ABLK:(c + 1) * ABLK],
                            wts[(kind, h)][:, d, :],
                            xTs[d][:, c0:c0 + ABLK],
                            start=(d == 0), stop=(d == ND - 1))
                        emitted += 1
                        if emitted % 32 == 0:
                            rr, hh_f = divmod(emitted // 32 - 1, 2)
                            kind2, h2 = b_rows[rr]
                            qk_row_drain(qp[(rr, hh_f)], kind2, h2, hh_f)
                    step_i += 1
                close_prev = make_close(u)
            close_prev()
            close_prev = None

            qkp_pool.release()
            xT_pool.release()
            wv_pool.release()
            wt_pool.release()
            # (xT/wv/wt are the top of the left SBUF stack, popped LIFO)

            # phase C: attention units (3,0),(2,2),(3,1),(2,3),(3,2),(3,3);
            # oproj matmuls flow from a pending queue (up to 3 per step),
            # with block b's oproj unlocked ~10 steps after head 3 block b
            # closes so the PE never parks on a normalize still in flight.
            wo_pool = tc.alloc_tile_pool(name="wo", bufs=1)
            wot = wo_pool.tile([P, NHC, DIM], BF16)
            nc.gpsimd.dma_start(wot[:], woT[:, :, :])
            ostage_pool = tc.alloc_tile_pool(name="ostage", bufs=4)

            with tc.tile_pool(name="op", bufs=2, space="PSUM",
                              side="right") as op_pool:
                op_state = {}

                def oproj_step(ti, half, hh, c):
                    """Emit matmul (hh, c) of oproj unit (ti, half)."""
                    if hh == 0 and c == 0:
                        op_state[(ti, half)] = op_pool.tile(
                            [P, HBLK], FP32, name=f"op{ti}_{half}", tag="op")
                    op = op_state[(ti, half)]
                    c0 = half * HBLK + c * ABLK
                    nc.tensor.matmul(op[:, c * ABLK:(c + 1) * ABLK],
                                     yT4[:, hh, ti * P:(ti + 1) * P],
                                     wot[:, hh, c0:c0 + ABLK],
                                     start=(hh == 0), stop=(hh == NHC - 1))
                    if hh == NHC - 1 and c == 1:
                        c0 = half * HBLK
                        os_ = ostage_pool.tile([P, HBLK], FP32,
                                               name=f"os{ti}_{half}", tag="os")
                        nc.vector.tensor_copy(out=os_[:], in_=op[:])
                        eng = nc.sync if (ti + half) % 2 == 0 else nc.gpsimd
                        eng.dma_start(
                            out[ti * P:(ti + 1) * P, c0:c0 + HBLK], os_[:])

                def oproj_block(b):
                    steps = []
                    for j in range(NTB):
                        for hf in range(2):
                            for hh in range(NHC):
                                for c in range(2):
                                    steps.append((b * NTB + j, hf, hh, c))
                    return steps

                c_units = [(1, 3), (2, 2), (3, 1), (2, 3), (3, 2), (3, 3)]
                pending = []
                # block-0 norms all land in B, except head 3 whose
                # normalize chain is still in flight at C start
                locked = [(6, oproj_block(0))]
                gstep = 0
                for h, b in c_units:
                    u = AttnUnit(h, b, st_pool, yacc_pool)
                    for i in range(NT):
                        u.step(i)
                        if i == 0 and close_prev is not None:
                            close_prev()
                        if i > 0:
                            u.av(i - 1)
                        for entry in locked[:]:
                            if entry[0] <= gstep:
                                pending.extend(entry[1])
                                locked.remove(entry)
                        for _ in range(3):
                            if pending:
                                oproj_step(*pending.pop(0))
                        gstep += 1
                    close_prev = make_close(u)
                    if h == 3 and b < NAB - 1:
                        locked.append((gstep + 10, oproj_block(b)))
                locked.append((0, oproj_block(NAB - 1)))
                close_prev()
                close_prev = None
                # tail: remaining oproj work (at least block 3)
                for entry in locked:
                    pending.extend(entry[1])
                for step in pending:
                    oproj_step(*step)

            ostage_pool.release()
            wo_pool.release()

        rs_pool.release()
        s3_pool.release()
        s2_pool.release()
        s1_pool.release()
        pt_pool.release()
        rope_pool.release()
        yT_pool.release()
        vh_pool.release()
        qk_pool.release()
        const_pool.release()

    nc.finalize()
    return nc


def _get_nc():
    if "nc" not in _CACHED:
        _CACHED["nc"] = _build()
    return _CACHED["nc"]


def _in_maps(x, wq, wk, wv, wo):
    import ml_dtypes

    BF = ml_dtypes.bfloat16
    tblc, tbls = _rope_tables()
    in_maps = []
    for core in range(8):
        b, g = divmod(core, 4)
        rows = slice(g * CC, (g + 1) * CC)
        in_maps.append({
            "xT": np.ascontiguousarray(x[b].T).astype(BF),
            "wqT": np.ascontiguousarray(
                wq[rows].reshape(NHC, HD, ND, P).transpose(0, 3, 2, 1)
            ).astype(BF),
            "wkT": np.ascontiguousarray(
                wk[rows].reshape(NHC, HD, ND, P).transpose(0, 3, 2, 1)
            ).astype(BF),
            "wvT": np.ascontiguousarray(wv[rows].T).astype(BF),
            "woT": np.ascontiguousarray(
                wo[:, rows].T.reshape(NHC, P, DIM).transpose(1, 0, 2)
            ).astype(BF),
            "tblc": tblc,
            "tbls": tbls,
        })
    return in_maps


def kernel(x, wq, wk, wv, wo):
    from concourse.bass_utils import run_bass_kernel_spmd

    x = np.asarray(x, dtype=np.float32)
    wq = np.asarray(wq, dtype=np.float32)
    wk = np.asarray(wk, dtype=np.float32)
    wv = np.asarray(wv, dtype=np.float32)
    wo = np.asarray(wo, dtype=np.float32)

    nc = _get_nc()
    res = run_bass_kernel_spmd(nc, _in_maps(x, wq, wk, wv, wo),
                               core_ids=list(range(8)))
    out = np.empty((B, T, DIM), dtype=np.float32)
    for b in range(B):
        out[b] = (res.results[4 * b + 0]["out"] + res.results[4 * b + 1]["out"]
                  + res.results[4 * b + 2]["out"] + res.results[4 * b + 3]["out"])
    return out
